# revision 24
# baseline (speedup 1.0000x reference)
"""GCN encoder (2x GCNConv + BN + ReLU + global mean pool) on 8 trn2 cores.

Self-contained: host-side sharding/prep + Bass program + SPMD run + unshard.

V6 plan (dst-sharded, 4 SWDGE queues, node-major window tiles):
  - Nodes permuted into 8*12544 slots; degree-balanced 128-dst windows.
  - Both layers gather 256B fp16 rows with dma_gather round-robin over the
    4 SWDGE queues (Q7 descriptor generation is the bottleneck; the 4 queue
    pairs work in parallel).
  - L1 gathers x rows (fp16 128 feats); per-window segment-sum via one-hot
    matmuls (S fp16 from DVE) -> psum [in, dst]; out1 = matmul(lhsT=seg,
    rhs=W1) -> [dst, hid] node-major.
  - BN stats per window via ones-vector matmuls (sum + sum-of-squares) into
    a [1, D] psum row, AllReduced; coef math on the row; a/bshift replicated
    to 128 partitions via a DRAM broadcast load; BN+ReLU applied per window
    on DVE; h staged node-major and AllGathered.
  - L2 gathers h PAIRS (two 64-dim fp16 nodes = 256B, idx = slot//2); parity
    via 256-wide one-hot (col = parity*128 + dstoff), two matmuls per chunk
    -> psum [hid, dst]; out2 = matmul(lhsT=seg2, rhs=W2) -> [dst, emb].
  - BN2 same row pattern; pooling via one-hot matmuls on the node-major
    applied tiles. Host sums per-core partials and divides by graph sizes.
"""
import heapq
from contextlib import ExitStack

import numpy as np

import concourse.bacc as bacc
import concourse.bass as bass
import concourse.mybir as mybir
from concourse.bass_utils import run_bass_kernel_spmd
from concourse.library_config import mlp

F32 = mybir.dt.float32
F16 = mybir.dt.float16
I16 = mybir.dt.int16
AF = mybir.ActivationFunctionType
OP = mybir.AluOpType

EPS = 1e-5
TRACE = False
NQ = 4          # SWDGE queues
NBUF = 8        # gather buffers
MAXCH = 8       # chunks per gather call (1024 idxs)

CFG_FULL = dict(n_nodes=100000, n_edges=1600000, n_cores=8,
                slots_per_core=12544, range_width=25088,
                in_dim=128, hid_dim=64, emb_dim=128, n_graphs=256)


# ================================================================ host prep
def _degree_balanced_perm(dst, n_nodes, n_windows, wsize):
    deg = np.bincount(dst, minlength=n_nodes)
    order = np.argsort(-deg, kind="stable")
    heap = [(0, w) for w in range(n_windows)]
    heapq.heapify(heap)
    counts = np.zeros(n_windows, np.int64)
    slot = np.empty(n_nodes, np.int64)
    degs = deg[order]
    for i in range(n_nodes):
        load, w = heapq.heappop(heap)
        slot[order[i]] = w * wsize + counts[w]
        counts[w] += 1
        if counts[w] < wsize:
            heapq.heappush(heap, (load + int(degs[i]), w))
    return slot


def _wrap16(flat):
    n = flat.size
    w = flat.reshape(n // 16, 16).T.astype(np.int16)
    return np.tile(w, (8, 1))


def _host_prep(x, edge_index, edge_weight, batch_vec, cfg):
    NC, SPC = cfg["n_cores"], cfg["slots_per_core"]
    W = 128
    NWC = SPC // W
    RW = cfg["range_width"]
    NR = (NC * SPC) // RW
    n_nodes = cfg["n_nodes"]

    src = np.asarray(edge_index[0], np.int64)
    dst = np.asarray(edge_index[1], np.int64)
    ew = np.asarray(edge_weight, np.float32)

    slot = _degree_balanced_perm(dst, n_nodes, NC * NWC, W)

    sslot, dslot = slot[src], slot[dst]
    core = dslot // SPC
    wloc = (dslot % SPC) // W
    dstoff = (dslot % W).astype(np.int64)
    rng = sslot // RW
    srel = sslot % RW

    key = (core * NWC + wloc) * NR + rng
    cnt = np.bincount(key, minlength=NC * NWC * NR).reshape(NC, NWC, NR)
    caps = np.maximum(128, ((cnt.max(axis=0) + 127) // 128) * 128)  # [NWC,NR]

    WG = 6
    groups = [list(range(s, min(s + WG, NWC))) for s in range(0, NWC, WG)]

    blocks, chunk_window, calls = [], [], []
    off = 0
    for g in groups:
        for r in range(NR):
            run_start = off
            for w in g:
                nch = int(caps[w][r]) // 128
                blocks.append((w, r, off))
                chunk_window.extend([w] * nch)
                off += nch
            k = run_start
            while k < off:
                n = min(MAXCH, off - k)
                calls.append((k, n, r))
                k += n
    n_chunks = off

    # per-core padded edge arrays, sorted into the static block layout
    idx1_cores, idx2_cores = [], []
    ew_cores, do1_cores, do2_cores = [], [], []
    for c in range(NC):
        m = core == c
        sr, dv, wv, rv, wgt = srel[m], dstoff[m], wloc[m], rng[m], ew[m]
        e_i1 = np.zeros(n_chunks * 128, np.int64)
        e_i2 = np.zeros(n_chunks * 128, np.int64)
        e_ew = np.zeros(n_chunks * 128, np.float32)
        e_d1 = np.zeros(n_chunks * 128, np.int64)
        e_d2 = np.zeros(n_chunks * 128, np.int64)
        for bi, (w, r, base) in enumerate(blocks):
            sel = (wv == w) & (rv == r)
            # evens first: most chunks become single-parity (narrow S)
            o = np.argsort(sr[sel] % 2, kind="stable")
            srl, dvl, wgl = sr[sel][o], dv[sel][o], wgt[sel][o]
            n = int(srl.size)
            s = base * 128
            e_i1[s:s + n] = srl
            e_i2[s:s + n] = srl // 2
            e_ew[s:s + n] = wgl
            e_d1[s:s + n] = dvl
            e_d2[s:s + n] = (srl % 2) * 128 + dvl
            # padding lanes keep idx 0 (safe row; ew=0 nulls them)
        idx1_cores.append(_wrap16(e_i1))
        idx2_cores.append(_wrap16(e_i2))
        ew_cores.append(np.ascontiguousarray(
            e_ew.reshape(n_chunks, 128).T))
        do1_cores.append(np.ascontiguousarray(
            e_d1.reshape(n_chunks, 128).T.astype(np.float32)))
        do2_cores.append(np.ascontiguousarray(
            e_d2.reshape(n_chunks, 128).T.astype(np.float32)))

    gid = np.full(NC * SPC, -1.0, np.float32)
    gid[slot] = np.asarray(batch_vec, np.float64).astype(np.float32)
    gid_cores = [np.ascontiguousarray(
        gid[c * SPC:(c + 1) * SPC].reshape(NWC, W).T)
        for c in range(NC)]

    xp16 = np.zeros((NC * SPC, x.shape[1]), np.float16)
    xp16[slot] = np.asarray(x, np.float32).astype(np.float16)

    # chunk parity kind shared across cores: 0 even-only, 1 odd-only, 2 mixed
    has_e = np.zeros(n_chunks, bool)
    has_o = np.zeros(n_chunks, bool)
    blk_end = {}
    for bi, (w, r, base) in enumerate(blocks):
        blk_end[bi] = base + int(caps[w][r]) // 128
    for c in range(NC):
        m = core == c
        sr, wv, rv = srel[m], wloc[m], rng[m]
        for bi, (w, r, base) in enumerate(blocks):
            sel = (wv == w) & (rv == r)
            pa = np.sort(sr[sel] % 2)
            n = int(pa.size)
            for t in range(base, blk_end[bi]):
                lo, hi = (t - base) * 128, min((t - base + 1) * 128, n)
                if lo >= n:
                    continue
                if (pa[lo:hi] == 0).any():
                    has_e[t] = True
                if (pa[lo:hi] == 1).any():
                    has_o[t] = True
    kind = np.where(has_e & has_o, 2, np.where(has_o, 1, 0))
    layout = dict(caps=caps, chunk_window=chunk_window, calls=calls,
                  n_chunks=n_chunks, NWC=NWC, NR=NR, WG=WG,
                  chunk_kind=kind.tolist())
    percore = dict(idx1=idx1_cores, idx2=idx2_cores, ew=ew_cores,
                   dof1=do1_cores, dof2=do2_cores, gid=gid_cores)
    return layout, percore, xp16, slot


# ============================================================= bass program
def _build(cfg, layout):
    NC, SPC = cfg["n_cores"], cfg["slots_per_core"]
    IN, HID, EMB = cfg["in_dim"], cfg["hid_dim"], cfg["emb_dim"]
    NG, RW = cfg["n_graphs"], cfg["range_width"]
    NSLOT = NC * SPC
    NWC, NR, WG = layout["NWC"], layout["NR"], layout["WG"]
    W = 128
    n_chunks = layout["n_chunks"]
    calls = layout["calls"]
    chunk_window = layout["chunk_window"]
    chunk_kind = layout["chunk_kind"]
    n_real = cfg["n_nodes"]
    GHALF = NG // 128

    wfirst, wlast = {}, {}
    for i, w in enumerate(chunk_window):
        wfirst.setdefault(w, i)
        wlast[w] = i
    worder = sorted(wlast, key=lambda w: wlast[w])
    wpos = {w: i for i, w in enumerate(worder)}

    cum_end = [cb + n for (cb, n, r) in calls]

    nc = bacc.Bacc("TRN2", num_swdge_queues=NQ)

    xp_d = nc.dram_tensor("xp16", [NSLOT, IN], F16, kind="ExternalInput")
    idx1_d = nc.dram_tensor("idx1", [128, n_chunks * 8], I16, kind="ExternalInput")
    idx2_d = nc.dram_tensor("idx2", [128, n_chunks * 8], I16, kind="ExternalInput")
    ew_d = nc.dram_tensor("ewt", [128, n_chunks], F32, kind="ExternalInput")
    do1_d = nc.dram_tensor("dof1", [128, n_chunks], F32, kind="ExternalInput")
    do2_d = nc.dram_tensor("dof2", [128, n_chunks], F32, kind="ExternalInput")
    gid_d = nc.dram_tensor("gid", [128, NWC], F32, kind="ExternalInput")
    w1_d = nc.dram_tensor("w1", [IN, HID], F16, kind="ExternalInput")
    w2_d = nc.dram_tensor("w2", [HID, EMB], F16, kind="ExternalInput")
    bn_d = nc.dram_tensor("bnp", [1, 6 * 128], F32, kind="ExternalInput")
    out_d = nc.dram_tensor("pool", [GHALF, 128, EMB], F32, kind="ExternalOutput")

    ag_in = nc.dram_tensor("ag_in", [SPC, HID], F16)
    ag_out = nc.dram_tensor("ag_out", [NSLOT, HID], F16, addr_space="Shared")
    ar1_in = nc.dram_tensor("ar1_in", [1, 2 * HID], F32)
    ar1_out = nc.dram_tensor("ar1_out", [1, 2 * HID], F32, addr_space="Shared")
    ar2_in = nc.dram_tensor("ar2_in", [1, 2 * EMB], F32)
    ar2_out = nc.dram_tensor("ar2_out", [1, 2 * EMB], F32, addr_space="Shared")
    cf1row_d = nc.dram_tensor("cf1row", [1, 2 * HID], F16)
    cf2row_d = nc.dram_tensor("cf2row", [1, 2 * EMB], F16)

    with ExitStack() as ctx:
        sb = lambda n, s, d: ctx.enter_context(nc.sbuf_tensor(n, s, d))
        sem = lambda n: ctx.enter_context(nc.semaphore(n))

        idx_sb = sb("idx_sb", [128, n_chunks * 8], I16)
        ew_sb = sb("ew_sb", [128, n_chunks], F32)
        do1_sb = sb("do1_sb", [128, n_chunks], F32)
        do2_sb = sb("do2_sb", [128, n_chunks], F32)
        gid_sb = sb("gid_sb", [128, NWC], F32)
        w1_sb = sb("w1_sb", [IN, HID], F16)
        w2_sb = sb("w2_sb", [HID, EMB], F16)
        bn_sb = sb("bn_sb", [1, 6 * 128], F32)
        iota128 = sb("iota128", [128, W], F32)
        iota256 = sb("iota256", [128, 2 * W], F32)
        iotg_sb = sb("iotg_sb", [128, NG], F32)
        ones_sb = sb("ones_sb", [128, 1], F16)

        mb = [sb(f"mb_{i}", [128, MAXCH, IN], F16) for i in range(NBUF)]
        NSLOT_S = 48
        s_sb = [sb(f"s_{i}", [128, 2 * W], F16) for i in range(NSLOT_S)]
        seg_sb = [sb(f"seg_{i}", [128, W], F16) for i in range(4)]
        sq_sb = [sb(f"sq_{i}", [128, EMB], F16) for i in range(4)]
        out1h_sb = sb("out1h_sb", [128, NWC * HID], F16)
        out2_sb = sb("out2_sb", [128, NWC * EMB], F16)
        gone_sb = [sb(f"gone_{i}", [128, NG], F16) for i in range(2)]
        pout_sb = sb("pout_sb", [128, GHALF * EMB], F32)
        strow_sb = sb("strow_sb", [1, 2 * EMB], F32)
        tmrow_sb = sb("tmrow_sb", [1, 2 * EMB], F32)
        cfrow_sb = sb("cfrow_sb", [1, 2 * EMB], F16)
        coefr_sb = sb("coefr_sb", [128, 2 * EMB], F16)

        # psum: banks 0-5 window tiles; bank 6 out1/out2; bank 7 pool+stats
        wseg = [ctx.enter_context(nc.psum_tensor(f"wseg{i}", [128, 512], F32))
                for i in range(WG)]
        b6 = ctx.enter_context(nc.psum_tensor("b6", [128, 512], F32))
        out1_ps = [b6[:, i * HID:(i + 1) * HID] for i in range(4)]
        out2_ps = [b6[:, 256:256 + EMB], b6[:, 256 + EMB:256 + 2 * EMB]]
        b7 = ctx.enter_context(nc.psum_tensor("b7", [128, 512], F32))
        pool_ps = [b7[:, i * EMB:(i + 1) * EMB] for i in range(GHALF)]
        st_ps = b7[:1, 2 * EMB:4 * EMB]       # [1, 2D] row: sums | sqsums

        io = sem("io")
        iox = sem("iox")
        ioz = sem("ioz")
        gs = [sem(f"gs_{b}") for b in range(NBUF)]
        sdone = sem("sdone")
        pchunk = sem("pchunk")
        segcp = sem("segcp")
        w1d = sem("w1d")
        ocp = sem("ocp")            # ACT window out-tile + square copies
        stm = sem("stm")            # PE stats matmul pairs
        stc = sem("stc")            # ACT stats psum->sbuf row copies
        arS, ar2S = sem("arS"), sem("ar2S")
        agS = sem("agS")
        cc = sem("cc")
        ar1L, ar2L = sem("ar1L"), sem("ar2L")
        cfa, cfb = sem("cfa"), sem("cfb")
        cf1, cf2 = sem("cf1"), sem("cf2")
        cfr = sem("cfr")
        hap = sem("hap")
        bn2r = sem("bn2r")
        gG = sem("gG")
        plm = sem("plm")
        outc = sem("outc")
        iot = sem("iot")
        cfc = sem("cfc")
        ioh = sem("ioh")

        NLOAD = 8        # input dma_starts
        cfc_n = [0]

        def _chain(v, inst):
            cfc_n[0] += 1
            inst.then_inc(cfc, 1)
            v.wait_ge(cfc, cfc_n[0])

        def _coef_math(v, D, ar_sem, phase, gcol, becol):
            # strow [1, 2D] = AllReduced (sum | sqsum) of raw out over nodes.
            # a = g / sqrt(var+eps); bshift = be - a*mu  (conv bias is zero)
            # -> cfrow [1, 2D] f16 = (a | bshift)
            v.wait_ge(ar_sem, 16)
            _chain(v, v.tensor_scalar_mul(tmrow_sb[:, :2 * D],
                                          strow_sb[:, :2 * D], 1.0 / n_real))
            # strow[0:D] = mu^2 ; strow[D:2D] = var + eps
            _chain(v, v.tensor_tensor(out=strow_sb[:, 0:D],
                                      in0=tmrow_sb[:, 0:D],
                                      in1=tmrow_sb[:, 0:D], op=OP.mult))
            _chain(v, v.tensor_tensor(out=strow_sb[:, D:2 * D],
                                      in0=tmrow_sb[:, D:2 * D],
                                      in1=strow_sb[:, 0:D], op=OP.subtract))
            v.tensor_scalar_add(strow_sb[:, D:2 * D], strow_sb[:, D:2 * D],
                                EPS).then_inc(cfa, 1)
            v.wait_ge(cfb, phase)        # ACT took sqrt in place
            _chain(v, v.reciprocal(out=strow_sb[:, D:2 * D],
                                   in_=strow_sb[:, D:2 * D]))
            # a (f32) -> strow[0:D]
            _chain(v, v.tensor_tensor(out=strow_sb[:, 0:D],
                                      in0=strow_sb[:, D:2 * D],
                                      in1=bn_sb[:, gcol * 128:gcol * 128 + D],
                                      op=OP.mult))
            # a*mu -> tmrow[0:D]; bshift (f32) -> tmrow[D:2D]
            _chain(v, v.tensor_tensor(out=tmrow_sb[:, 0:D],
                                      in0=strow_sb[:, 0:D],
                                      in1=tmrow_sb[:, 0:D], op=OP.mult))
            _chain(v, v.tensor_tensor(out=tmrow_sb[:, D:2 * D],
                                      in0=bn_sb[:, becol * 128:becol * 128 + D],
                                      in1=tmrow_sb[:, 0:D], op=OP.subtract))
            # cast to f16 row (a | bshift)
            _chain(v, v.tensor_copy(out=cfrow_sb[:, 0:D],
                                    in_=strow_sb[:, 0:D]))
            v.tensor_copy(out=cfrow_sb[:, D:2 * D],
                          in_=tmrow_sb[:, D:2 * D]
                          ).then_inc(cf1 if phase == 1 else cf2, 1)

        with nc.Block() as block:

            # ------------------------------------------------ GPSIMD
            @block.gpsimd
            def _(gp: bass.BassGpSimd):
                gp.load_library(mlp)
                for dst_ap, src_ap in (
                    (idx_sb[:, :], idx1_d[:, :]),
                    (ew_sb[:, :], ew_d[:, :]),
                    (do1_sb[:, :], do1_d[:, :]),
                    (do2_sb[:, :], do2_d[:, :]),
                    (gid_sb[:, :], gid_d[:, :]),
                    (w1_sb[:, :], w1_d[:, :]),
                    (w2_sb[:, :], w2_d[:, :]),
                    (bn_sb[:, :], bn_d[:, :]),
                ):
                    gp.dma_start(dst_ap, src_ap).then_inc(io, 16)
                # zero gather buffers once (stale lanes must be finite)
                for i in range(NBUF):
                    gp.memset(mb[i][:, :, :], 0.0).then_inc(ioz, 1)
                gp.memset(ones_sb[:, :], 1.0).then_inc(ioz, 1)
                gp.wait_ge(ioz, NBUF + 1)
                gp.wait_ge(io, 16 * NLOAD)
                gp.iota(iota128[:, :], [[1, W]], base=0, channel_multiplier=0,
                        allow_small_or_imprecise_dtypes=True)
                gp.iota(iota256[:, :], [[1, 2 * W]], base=0,
                        channel_multiplier=0,
                        allow_small_or_imprecise_dtypes=True)
                gp.iota(iotg_sb[:, :], [[1, NG]], base=0, channel_multiplier=0,
                        allow_small_or_imprecise_dtypes=True).then_inc(iot, 1)

                # layer-1 gathers
                for ci, (cb, nch, r) in enumerate(calls):
                    q = ci % NQ
                    b = ci % NBUF
                    if ci >= NBUF:
                        gp.wait_ge(pchunk, cum_end[ci - NBUF])
                    nidx = nch * 128
                    gp.dma_gather(
                        mb[b][:, :nch, :], xp_d[r * RW:(r + 1) * RW, :],
                        idx_sb[:, cb * 8:cb * 8 + nidx // 16],
                        nidx, nidx, IN, queue_num=q,
                    ).then_inc(gs[b], 16)

                # BN1 stats AllReduce
                gp.wait_ge(arS, 16)
                gp.collective_compute(
                    "AllReduce", OP.add, replica_groups=[list(range(NC))],
                    ins=[ar1_in[:, :]], outs=[ar1_out[:, :]]).then_inc(cc, 1)

                # idx2 reload (L1 gathers all done once stats staged)
                gp.dma_start(idx_sb[:, :], idx2_d[:, :]).then_inc(iox, 16)

                # h AllGather
                gp.wait_ge(agS, 16)
                gp.collective_compute(
                    "AllGather", OP.bypass, replica_groups=[list(range(NC))],
                    ins=[ag_in[:, :]], outs=[ag_out[:, :]]).then_inc(cc, 1)

                # layer-2 gathers (h pairs from ag_out)
                gp.wait_ge(iox, 16)
                gp.wait_ge(cc, 2)
                for ci, (cb, nch, r) in enumerate(calls):
                    q = ci % NQ
                    b = ci % NBUF
                    if ci >= NBUF:
                        gp.wait_ge(pchunk, n_chunks + cum_end[ci - NBUF])
                    else:
                        gp.wait_ge(pchunk, n_chunks)
                    nidx = nch * 128
                    gp.dma_gather(
                        mb[b][:, :nch, :],
                        ag_out[r * RW:(r + 1) * RW, :]
                        .rearrange("(a b) d -> a (b d)", b=2),
                        idx_sb[:, cb * 8:cb * 8 + nidx // 16],
                        nidx, nidx, EMB, queue_num=q,
                    ).then_inc(gs[b], 16)

                # BN2 stats AllReduce
                gp.wait_ge(ar2S, 16)
                gp.collective_compute(
                    "AllReduce", OP.add, replica_groups=[list(range(NC))],
                    ins=[ar2_in[:, :]], outs=[ar2_out[:, :]]).then_inc(cc, 1)

            # ------------------------------------------------ VECTOR
            @block.vector
            def _(v):
                v.wait_ge(io, 16 * NLOAD)
                v.wait_ge(iot, 1)

                # L1 one-hot S
                for (cb, nch, r) in calls:
                    if cb + nch > NSLOT_S:
                        v.wait_ge(pchunk, cb + nch - NSLOT_S)
                    for t in range(cb, cb + nch):
                        v.tensor_scalar(
                            out=s_sb[t % NSLOT_S][:, :W], in0=iota128[:, :],
                            scalar1=do1_sb[:, t:t + 1],
                            scalar2=ew_sb[:, t:t + 1],
                            op0=OP.is_equal, op1=OP.mult).then_inc(sdone, 1)

                # BN1 coef (stats row AllReduced by now)
                _coef_math(v, HID, ar1L, 1, 1, 2)
                # BN1 apply per window on node-major tiles + ReLU
                v.wait_ge(cfr, 48)
                for wi in range(NWC):
                    w = worder[wi]
                    sl = slice(w * HID, (w + 1) * HID)
                    _chain(v, v.tensor_tensor(
                        out=out1h_sb[:, sl], in0=out1h_sb[:, sl],
                        in1=coefr_sb[:, 0:HID], op=OP.mult))
                    _chain(v, v.tensor_tensor(
                        out=out1h_sb[:, sl], in0=out1h_sb[:, sl],
                        in1=coefr_sb[:, EMB:EMB + HID], op=OP.add))
                    v.tensor_scalar_max(
                        out=out1h_sb[:, sl], in0=out1h_sb[:, sl],
                        scalar1=0.0).then_inc(hap, 1)

                # L2 one-hot S: narrow for single-parity chunks, 256-wide
                # (col = parity*128 + dstoff) for mixed chunks
                for (cb, nch, r) in calls:
                    v.wait_ge(pchunk, n_chunks + max(0, cb + nch - NSLOT_S))
                    for t in range(cb, cb + nch):
                        if chunk_kind[t] == 2:
                            v.tensor_scalar(
                                out=s_sb[t % NSLOT_S][:, :], in0=iota256[:, :],
                                scalar1=do2_sb[:, t:t + 1],
                                scalar2=ew_sb[:, t:t + 1],
                                op0=OP.is_equal, op1=OP.mult).then_inc(sdone, 1)
                        else:
                            v.tensor_scalar(
                                out=s_sb[t % NSLOT_S][:, :W], in0=iota128[:, :],
                                scalar1=do1_sb[:, t:t + 1],
                                scalar2=ew_sb[:, t:t + 1],
                                op0=OP.is_equal, op1=OP.mult).then_inc(sdone, 1)

                # BN2 coef
                _coef_math(v, EMB, ar2L, 2, 4, 5)
                # BN2 apply per window + ReLU; gone one-hots
                v.wait_ge(cfr, 80)
                for wi in range(NWC):
                    w = worder[wi]
                    sl = slice(w * EMB, (w + 1) * EMB)
                    _chain(v, v.tensor_tensor(
                        out=out2_sb[:, sl], in0=out2_sb[:, sl],
                        in1=coefr_sb[:, 0:EMB], op=OP.mult))
                    _chain(v, v.tensor_tensor(
                        out=out2_sb[:, sl], in0=out2_sb[:, sl],
                        in1=coefr_sb[:, EMB:2 * EMB], op=OP.add))
                    v.tensor_scalar_max(
                        out=out2_sb[:, sl], in0=out2_sb[:, sl],
                        scalar1=0.0).then_inc(bn2r, 1)
                    if wi >= 2:
                        v.wait_ge(plm, wi - 1)
                    v.tensor_scalar(
                        out=gone_sb[wi % 2][:, :], in0=iotg_sb[:, :],
                        scalar1=gid_sb[:, w:w + 1], scalar2=None,
                        op0=OP.is_equal).then_inc(gG, 1)

            # ------------------------------------------------ SCALAR
            @block.scalar
            def _(sc):
                sc.wait_ge(io, 16 * NLOAD)
                for L in range(2):
                    D = HID if L == 0 else EMB
                    osb = out1h_sb if L == 0 else out2_sb
                    ops = out1_ps if L == 0 else out2_ps
                    NOUT = len(ops)
                    # per-window drain: seg copy (for W matmul), out copy + sq
                    for wi in range(NWC):
                        w = worder[wi]
                        sc.wait_ge(pchunk, L * n_chunks + wlast[w] + 1)
                        if wi >= 4:
                            sc.wait_ge(w1d, L * NWC + wi - 3)
                        if L == 0:
                            sc.activation(out=seg_sb[wi % 4][:, :],
                                          in_=wseg[wi % WG][:, :W],
                                          func=AF.Copy).then_inc(segcp, 1)
                        else:
                            sc.activation(out=seg_sb[wi % 4][:HID, :],
                                          in_=wseg[wi % WG][:HID, :W],
                                          func=AF.Copy).then_inc(segcp, 1)
                        sc.wait_ge(w1d, L * NWC + wi + 1)
                        if wi >= 4:
                            sc.wait_ge(stm, L * NWC + wi - 3)
                        sc.activation(out=osb[:, w * D:(w + 1) * D],
                                      in_=ops[wi % NOUT][:, :D], func=AF.Copy)
                        sc.activation(out=sq_sb[wi % 4][:, :D],
                                      in_=ops[wi % NOUT][:, :D],
                                      func=AF.Square).then_inc(ocp, 1)
                    # stats row psum -> sbuf
                    sc.wait_ge(stm, (L + 1) * NWC)
                    sc.activation(out=strow_sb[:, :2 * D],
                                  in_=st_ps[:, :2 * D],
                                  func=AF.Copy).then_inc(stc, 1)
                    # sqrt for BN coef
                    sc.wait_ge(cfa, L + 1)
                    sc.activation(out=strow_sb[:, D:2 * D],
                                  in_=strow_sb[:, D:2 * D],
                                  func=AF.Sqrt).then_inc(cfb, 1)
                # final pool copy
                sc.wait_ge(plm, NWC)
                sc.activation(out=pout_sb[:, :], in_=b7[:, :GHALF * EMB],
                              func=AF.Copy).then_inc(outc, 1)

            # ------------------------------------------------ TENSOR
            @block.tensor
            def _(pe):
                pe.wait_ge(io, 16 * NLOAD)
                uses = [0] * NBUF

                for L in range(2):
                    D = HID if L == 0 else EMB
                    osb = out1h_sb if L == 0 else out2_sb
                    ops = out1_ps if L == 0 else out2_ps
                    NOUT = len(ops)
                    done_w = 0
                    done_st = 0

                    def drain(upto_w, upto_st, L=L, D=D, osb=osb, ops=ops,
                              NOUT=None):
                        nonlocal done_w, done_st
                        NOUT = len(ops)
                        while done_w < upto_w:
                            wi = done_w
                            pe.wait_ge(segcp, L * NWC + wi + 1)
                            if wi >= NOUT:
                                pe.wait_ge(ocp, L * NWC + wi - NOUT + 1)
                            if L == 0:
                                pe.matmul(ops[wi % NOUT][:, :HID],
                                          lhsT=seg_sb[wi % 4][:, :],
                                          rhs=w1_sb[:, :],
                                          start=True, stop=True
                                          ).then_inc(w1d, 1)
                            else:
                                pe.matmul(ops[wi % NOUT][:, :EMB],
                                          lhsT=seg_sb[wi % 4][:HID, :],
                                          rhs=w2_sb[:, :],
                                          start=True, stop=True
                                          ).then_inc(w1d, 1)
                            done_w += 1
                        while done_st < upto_st:
                            wi = done_st
                            w = worder[wi]
                            pe.wait_ge(ocp, L * NWC + wi + 1)
                            pe.matmul(st_ps[:, 0:D],
                                      lhsT=ones_sb[:, :],
                                      rhs=osb[:, w * D:(w + 1) * D],
                                      start=(wi == 0),
                                      stop=False)
                            pe.matmul(st_ps[:, D:2 * D],
                                      lhsT=ones_sb[:, :],
                                      rhs=sq_sb[wi % 4][:, :D],
                                      start=False,
                                      stop=(wi == NWC - 1)).then_inc(stm, 1)
                            done_st += 1

                    seen_w = 0
                    for ci, (cb, nch, r) in enumerate(calls):
                        b = ci % NBUF
                        uses[b] += 1
                        pe.wait_ge(gs[b], 16 * uses[b])
                        pe.wait_ge(sdone, L * n_chunks + cb + nch)
                        for k in range(nch):
                            t = cb + k
                            w = chunk_window[t]
                            wi = wpos[w]
                            if t == wfirst[w] and wi >= WG:
                                pe.wait_ge(segcp, L * NWC + wi - WG + 1)
                            first, lastc = t == wfirst[w], t == wlast[w]
                            if L == 0:
                                pe.matmul(wseg[wi % WG][:, :W],
                                          lhsT=mb[b][:, k, :],
                                          rhs=s_sb[t % NSLOT_S][:, :W],
                                          start=first,
                                          stop=lastc).then_inc(pchunk, 1)
                            elif chunk_kind[t] == 2:
                                pe.matmul(wseg[wi % WG][:HID, :W],
                                          lhsT=mb[b][:, k, 0:HID],
                                          rhs=s_sb[t % NSLOT_S][:, 0:W],
                                          start=first, stop=False)
                                pe.matmul(wseg[wi % WG][:HID, :W],
                                          lhsT=mb[b][:, k, HID:2 * HID],
                                          rhs=s_sb[t % NSLOT_S][:, W:2 * W],
                                          start=False,
                                          stop=lastc).then_inc(pchunk, 1)
                            else:
                                hh = chunk_kind[t] * HID
                                pe.matmul(wseg[wi % WG][:HID, :W],
                                          lhsT=mb[b][:, k, hh:hh + HID],
                                          rhs=s_sb[t % NSLOT_S][:, :W],
                                          start=first,
                                          stop=lastc).then_inc(pchunk, 1)
                            if lastc:
                                seen_w += 1
                                drain(max(0, seen_w - 1), max(0, seen_w - 2))
                    drain(NWC, NWC)

                # pool matmuls on BN2-applied node-major tiles
                for wi in range(NWC):
                    w = worder[wi]
                    pe.wait_ge(bn2r, wi + 1)
                    pe.wait_ge(gG, wi + 1)
                    for gh in range(GHALF):
                        mm = pe.matmul(
                            pool_ps[gh][:, :],
                            lhsT=gone_sb[wi % 2][:, gh * 128:(gh + 1) * 128],
                            rhs=out2_sb[:, w * EMB:(w + 1) * EMB],
                            start=(wi == 0 and gh == 0),
                            stop=(wi == NWC - 1 and gh == GHALF - 1))
                        if gh == GHALF - 1:
                            mm.then_inc(plm, 1)

            # ------------------------------------------------ SYNC
            @block.sync
            def _(sy):
                # BN1 stats row out / in
                sy.wait_ge(stc, 1)
                sy.dma_start(ar1_in[:, :], strow_sb[:, :2 * HID]).then_inc(arS, 16)
                sy.wait_ge(cc, 1)
                sy.dma_start(strow_sb[:, :2 * HID], ar1_out[:, :]).then_inc(ar1L, 16)
                # BN1 coef row -> DRAM -> replicated rows
                sy.wait_ge(cf1, 1)
                sy.dma_start(cf1row_d[:, :], cfrow_sb[:, :2 * HID]).then_inc(cfr, 16)
                sy.wait_ge(cfr, 16)
                rep1 = bass.AP(cf1row_d, 0, [[0, 128], [1, 2 * HID]])
                sy.dma_start(coefr_sb[:, 0:HID], rep1[:, 0:HID]).then_inc(cfr, 16)
                sy.dma_start(coefr_sb[:, EMB:EMB + HID],
                             rep1[:, HID:2 * HID]).then_inc(cfr, 16)
                # h to AllGather input (BN1-applied, node-major)
                sy.wait_ge(hap, NWC)
                sy.dma_start(
                    ag_in[:, :].rearrange("(w p) c -> p w c", p=128),
                    out1h_sb[:, :].rearrange("p (w c) -> p w c", c=HID),
                ).then_inc(agS, 16)
                # BN2 stats row
                sy.wait_ge(stc, 2)
                sy.dma_start(ar2_in[:, :], strow_sb[:, :2 * EMB]).then_inc(ar2S, 16)
                sy.wait_ge(cc, 3)
                sy.dma_start(strow_sb[:, :2 * EMB], ar2_out[:, :]).then_inc(ar2L, 16)
                # BN2 coef row -> DRAM -> replicated rows
                sy.wait_ge(cf2, 1)
                sy.dma_start(cf2row_d[:, :], cfrow_sb[:, :2 * EMB]).then_inc(cfr, 16)
                sy.wait_ge(cfr, 64)
                rep2 = bass.AP(cf2row_d, 0, [[0, 128], [1, 2 * EMB]])
                sy.dma_start(coefr_sb[:, :], rep2[:, :]).then_inc(cfr, 16)
                # final output
                sy.wait_ge(outc, 1)
                sy.dma_start(
                    out_d[:, :, :].rearrange("g p d -> p g d"),
                    pout_sb[:, :].rearrange("p (g d) -> p g d", d=EMB),
                ).then_inc(ioh, 16)
                sy.wait_ge(ioh, 16)

    nc.compile()
    return nc


# ==================================================================== entry
def _make_in_maps(inputs, cfg, percore, xp16):
    HID, EMB = cfg["hid_dim"], cfg["emb_dim"]
    bnp = np.zeros((1, 6 * 128), np.float32)
    bnp[0, 0:HID] = np.asarray(inputs["b1"], np.float32)
    bnp[0, 128:128 + HID] = np.asarray(inputs["g1"], np.float32)
    bnp[0, 256:256 + HID] = np.asarray(inputs["be1"], np.float32)
    bnp[0, 384:384 + EMB] = np.asarray(inputs["b2"], np.float32)
    bnp[0, 512:512 + EMB] = np.asarray(inputs["g2"], np.float32)
    bnp[0, 640:640 + EMB] = np.asarray(inputs["be2"], np.float32)
    w1 = np.asarray(inputs["W1"], np.float32).astype(np.float16)
    w2 = np.asarray(inputs["W2"], np.float32).astype(np.float16)
    return [dict(
        xp16=xp16, idx1=percore["idx1"][c], idx2=percore["idx2"][c],
        ewt=percore["ew"][c], dof1=percore["dof1"][c],
        dof2=percore["dof2"][c], gid=percore["gid"][c],
        w1=w1, w2=w2, bnp=bnp,
    ) for c in range(cfg["n_cores"])]


def _run(inputs, cfg):
    x = np.asarray(inputs["x"], np.float32)
    layout, percore, xp16, slot = _host_prep(
        x, inputs["edge_index"], inputs["edge_weight"], inputs["batch_vec"], cfg)
    nc = _build(cfg, layout)

    NC = cfg["n_cores"]
    in_maps = _make_in_maps(inputs, cfg, percore, xp16)
    res = None
    last_err = None
    for attempt in range(3):
        try:
            res = run_bass_kernel_spmd(nc, in_maps, list(range(NC)), trace=TRACE)
            break
        except Exception as err:      # wedged device: retry
            last_err = err
    if res is None:
        raise last_err

    NG, EMB = cfg["n_graphs"], cfg["emb_dim"]
    pool = np.zeros((NG, EMB), np.float64)
    for c in range(NC):
        p = res.results[c]["pool"].astype(np.float64)   # [GHALF, 128, EMB]
        pool += p.reshape(NG, EMB)
    counts = np.bincount(np.asarray(inputs["batch_vec"], np.int64),
                         minlength=NG).astype(np.float64)
    pool /= np.maximum(counts, 1.0)[:, None]
    return pool.astype(np.float32), res


def kernel(**inputs):
    out, _ = _run(inputs, CFG_FULL)
    return out


# revision 26
# speedup vs baseline: 1.0032x; 1.0032x over previous
"""GCN encoder (2x GCNConv + BN + ReLU + global mean pool) on 8 trn2 cores.

Self-contained: host-side sharding/prep + Bass program + SPMD run + unshard.

V6 plan (dst-sharded, 4 SWDGE queues, node-major window tiles):
  - Nodes permuted into 8*12544 slots; degree-balanced 128-dst windows.
  - Both layers gather 256B fp16 rows with dma_gather round-robin over the
    4 SWDGE queues (Q7 descriptor generation is the bottleneck; the 4 queue
    pairs work in parallel).
  - L1 gathers x rows (fp16 128 feats); per-window segment-sum via one-hot
    matmuls (S fp16 from DVE) -> psum [in, dst]; out1 = matmul(lhsT=seg,
    rhs=W1) -> [dst, hid] node-major.
  - BN stats per window via ones-vector matmuls (sum + sum-of-squares) into
    a [1, D] psum row, AllReduced; coef math on the row; a/bshift replicated
    to 128 partitions via a DRAM broadcast load; BN+ReLU applied per window
    on DVE; h staged node-major and AllGathered.
  - L2 gathers h PAIRS (two 64-dim fp16 nodes = 256B, idx = slot//2); parity
    via 256-wide one-hot (col = parity*128 + dstoff), two matmuls per chunk
    -> psum [hid, dst]; out2 = matmul(lhsT=seg2, rhs=W2) -> [dst, emb].
  - BN2 same row pattern; pooling via one-hot matmuls on the node-major
    applied tiles. Host sums per-core partials and divides by graph sizes.
"""
import heapq
from contextlib import ExitStack

import numpy as np

import concourse.bacc as bacc
import concourse.bass as bass
import concourse.mybir as mybir
from concourse.bass_utils import run_bass_kernel_spmd
from concourse.library_config import mlp

F32 = mybir.dt.float32
F16 = mybir.dt.float16
I16 = mybir.dt.int16
AF = mybir.ActivationFunctionType
OP = mybir.AluOpType

EPS = 1e-5
TRACE = False
NQ = 4          # SWDGE queues
NBUF = 8        # gather buffers
MAXCH = 8       # chunks per gather call (1024 idxs)

CFG_FULL = dict(n_nodes=100000, n_edges=1600000, n_cores=8,
                slots_per_core=12544, range_width=25088,
                in_dim=128, hid_dim=64, emb_dim=128, n_graphs=256)


# ================================================================ host prep
def _degree_balanced_perm(dst, n_nodes, n_windows, wsize):
    deg = np.bincount(dst, minlength=n_nodes)
    order = np.argsort(-deg, kind="stable")
    heap = [(0, w) for w in range(n_windows)]
    heapq.heapify(heap)
    counts = np.zeros(n_windows, np.int64)
    slot = np.empty(n_nodes, np.int64)
    degs = deg[order]
    for i in range(n_nodes):
        load, w = heapq.heappop(heap)
        slot[order[i]] = w * wsize + counts[w]
        counts[w] += 1
        if counts[w] < wsize:
            heapq.heappush(heap, (load + int(degs[i]), w))
    return slot


def _wrap16(flat):
    n = flat.size
    w = flat.reshape(n // 16, 16).T.astype(np.int16)
    return np.tile(w, (8, 1))


def _host_prep(x, edge_index, edge_weight, batch_vec, cfg):
    NC, SPC = cfg["n_cores"], cfg["slots_per_core"]
    W = 128
    NWC = SPC // W
    RW = cfg["range_width"]
    NR = (NC * SPC) // RW
    n_nodes = cfg["n_nodes"]

    src = np.asarray(edge_index[0], np.int64)
    dst = np.asarray(edge_index[1], np.int64)
    ew = np.asarray(edge_weight, np.float32)

    slot = _degree_balanced_perm(dst, n_nodes, NC * NWC, W)

    sslot, dslot = slot[src], slot[dst]
    core = dslot // SPC
    wloc = (dslot % SPC) // W
    dstoff = (dslot % W).astype(np.int64)
    rng = sslot // RW
    srel = sslot % RW

    key = (core * NWC + wloc) * NR + rng
    cnt = np.bincount(key, minlength=NC * NWC * NR).reshape(NC, NWC, NR)
    caps = np.maximum(128, ((cnt.max(axis=0) + 127) // 128) * 128)  # [NWC,NR]

    WG = 6
    groups = [list(range(s, min(s + WG, NWC))) for s in range(0, NWC, WG)]

    blocks, chunk_window, calls = [], [], []
    off = 0
    for g in groups:
        for r in range(NR):
            run_start = off
            for w in g:
                nch = int(caps[w][r]) // 128
                blocks.append((w, r, off))
                chunk_window.extend([w] * nch)
                off += nch
            k = run_start
            while k < off:
                n = min(MAXCH, off - k)
                calls.append((k, n, r))
                k += n
    n_chunks = off

    # per-core padded edge arrays, sorted into the static block layout
    idx1_cores, idx2_cores = [], []
    ew_cores, do1_cores, do2_cores = [], [], []
    for c in range(NC):
        m = core == c
        sr, dv, wv, rv, wgt = srel[m], dstoff[m], wloc[m], rng[m], ew[m]
        e_i1 = np.zeros(n_chunks * 128, np.int64)
        e_i2 = np.zeros(n_chunks * 128, np.int64)
        e_ew = np.zeros(n_chunks * 128, np.float32)
        e_d1 = np.zeros(n_chunks * 128, np.int64)
        e_d2 = np.zeros(n_chunks * 128, np.int64)
        for bi, (w, r, base) in enumerate(blocks):
            sel = (wv == w) & (rv == r)
            # evens first: most chunks become single-parity (narrow S)
            o = np.argsort(sr[sel] % 2, kind="stable")
            srl, dvl, wgl = sr[sel][o], dv[sel][o], wgt[sel][o]
            n = int(srl.size)
            s = base * 128
            e_i1[s:s + n] = srl
            e_i2[s:s + n] = srl // 2
            e_ew[s:s + n] = wgl
            e_d1[s:s + n] = dvl
            e_d2[s:s + n] = (srl % 2) * 128 + dvl
            # padding lanes keep idx 0 (safe row; ew=0 nulls them)
        idx1_cores.append(_wrap16(e_i1))
        idx2_cores.append(_wrap16(e_i2))
        ew_cores.append(np.ascontiguousarray(
            e_ew.reshape(n_chunks, 128).T))
        do1_cores.append(np.ascontiguousarray(
            e_d1.reshape(n_chunks, 128).T.astype(np.float32)))
        do2_cores.append(np.ascontiguousarray(
            e_d2.reshape(n_chunks, 128).T.astype(np.float32)))

    gid = np.full(NC * SPC, -1.0, np.float32)
    gid[slot] = np.asarray(batch_vec, np.float64).astype(np.float32)
    gid_cores = [np.ascontiguousarray(
        gid[c * SPC:(c + 1) * SPC].reshape(NWC, W).T)
        for c in range(NC)]

    xp16 = np.zeros((NC * SPC, x.shape[1]), np.float16)
    xp16[slot] = np.asarray(x, np.float32).astype(np.float16)

    # chunk parity kind shared across cores: 0 even-only, 1 odd-only, 2 mixed
    has_e = np.zeros(n_chunks, bool)
    has_o = np.zeros(n_chunks, bool)
    blk_end = {}
    for bi, (w, r, base) in enumerate(blocks):
        blk_end[bi] = base + int(caps[w][r]) // 128
    for c in range(NC):
        m = core == c
        sr, wv, rv = srel[m], wloc[m], rng[m]
        for bi, (w, r, base) in enumerate(blocks):
            sel = (wv == w) & (rv == r)
            pa = np.sort(sr[sel] % 2)
            n = int(pa.size)
            for t in range(base, blk_end[bi]):
                lo, hi = (t - base) * 128, min((t - base + 1) * 128, n)
                if lo >= n:
                    continue
                if (pa[lo:hi] == 0).any():
                    has_e[t] = True
                if (pa[lo:hi] == 1).any():
                    has_o[t] = True
    kind = np.where(has_e & has_o, 2, np.where(has_o, 1, 0))
    layout = dict(caps=caps, chunk_window=chunk_window, calls=calls,
                  n_chunks=n_chunks, NWC=NWC, NR=NR, WG=WG,
                  chunk_kind=kind.tolist())
    percore = dict(idx1=idx1_cores, idx2=idx2_cores, ew=ew_cores,
                   dof1=do1_cores, dof2=do2_cores, gid=gid_cores)
    return layout, percore, xp16, slot


# ============================================================= bass program
def _build(cfg, layout):
    NC, SPC = cfg["n_cores"], cfg["slots_per_core"]
    IN, HID, EMB = cfg["in_dim"], cfg["hid_dim"], cfg["emb_dim"]
    NG, RW = cfg["n_graphs"], cfg["range_width"]
    NSLOT = NC * SPC
    NWC, NR, WG = layout["NWC"], layout["NR"], layout["WG"]
    W = 128
    n_chunks = layout["n_chunks"]
    calls = layout["calls"]
    chunk_window = layout["chunk_window"]
    chunk_kind = layout["chunk_kind"]
    n_real = cfg["n_nodes"]
    GHALF = NG // 128

    wfirst, wlast = {}, {}
    for i, w in enumerate(chunk_window):
        wfirst.setdefault(w, i)
        wlast[w] = i
    worder = sorted(wlast, key=lambda w: wlast[w])
    wpos = {w: i for i, w in enumerate(worder)}

    cum_end = [cb + n for (cb, n, r) in calls]

    nc = bacc.Bacc("TRN2", num_swdge_queues=NQ)

    xp_d = nc.dram_tensor("xp16", [NSLOT, IN], F16, kind="ExternalInput")
    idx1_d = nc.dram_tensor("idx1", [128, n_chunks * 8], I16, kind="ExternalInput")
    idx2_d = nc.dram_tensor("idx2", [128, n_chunks * 8], I16, kind="ExternalInput")
    ew_d = nc.dram_tensor("ewt", [128, n_chunks], F32, kind="ExternalInput")
    do1_d = nc.dram_tensor("dof1", [128, n_chunks], F32, kind="ExternalInput")
    do2_d = nc.dram_tensor("dof2", [128, n_chunks], F32, kind="ExternalInput")
    gid_d = nc.dram_tensor("gid", [128, NWC], F32, kind="ExternalInput")
    w1_d = nc.dram_tensor("w1", [IN, HID], F16, kind="ExternalInput")
    w2_d = nc.dram_tensor("w2", [HID, EMB], F16, kind="ExternalInput")
    bn_d = nc.dram_tensor("bnp", [1, 6 * 128], F32, kind="ExternalInput")
    out_d = nc.dram_tensor("pool", [GHALF, 128, EMB], F32, kind="ExternalOutput")

    ag_in = nc.dram_tensor("ag_in", [SPC, HID], F16)
    ag_out = nc.dram_tensor("ag_out", [NSLOT, HID], F16, addr_space="Shared")
    ar1_in = nc.dram_tensor("ar1_in", [1, 2 * HID], F32)
    ar1_out = nc.dram_tensor("ar1_out", [1, 2 * HID], F32, addr_space="Shared")
    ar2_in = nc.dram_tensor("ar2_in", [1, 2 * EMB], F32)
    ar2_out = nc.dram_tensor("ar2_out", [1, 2 * EMB], F32, addr_space="Shared")
    cf1row_d = nc.dram_tensor("cf1row", [1, 2 * HID], F16)
    cf2row_d = nc.dram_tensor("cf2row", [1, 2 * EMB], F16)

    with ExitStack() as ctx:
        sb = lambda n, s, d: ctx.enter_context(nc.sbuf_tensor(n, s, d))
        sem = lambda n: ctx.enter_context(nc.semaphore(n))

        idx_sb = sb("idx_sb", [128, n_chunks * 8], I16)
        ew_sb = sb("ew_sb", [128, n_chunks], F32)
        do1_sb = sb("do1_sb", [128, n_chunks], F32)
        do2_sb = sb("do2_sb", [128, n_chunks], F32)
        gid_sb = sb("gid_sb", [128, NWC], F32)
        w1_sb = sb("w1_sb", [IN, HID], F16)
        w2_sb = sb("w2_sb", [HID, EMB], F16)
        bn_sb = sb("bn_sb", [1, 6 * 128], F32)
        iota128 = sb("iota128", [128, W], F32)
        iota256 = sb("iota256", [128, 2 * W], F32)
        iotg_sb = sb("iotg_sb", [128, NG], F32)
        ones_sb = sb("ones_sb", [128, 1], F16)

        mb = [sb(f"mb_{i}", [128, MAXCH, IN], F16) for i in range(NBUF)]
        NSLOT_S = 48
        s_sb = [sb(f"s_{i}", [128, 2 * W], F16) for i in range(NSLOT_S)]
        seg_sb = [sb(f"seg_{i}", [128, W], F16) for i in range(2)]
        sq_sb = [sb(f"sq_{i}", [128, EMB], F16) for i in range(2)]
        out1h_sb = sb("out1h_sb", [128, NWC * HID], F16)
        out2_sb = sb("out2_sb", [128, NWC * EMB], F16)
        gone_sb = [sb(f"gone_{i}", [128, NG], F16) for i in range(2)]
        pout_sb = sb("pout_sb", [128, GHALF * EMB], F32)
        strow_sb = sb("strow_sb", [1, 2 * EMB], F32)
        tmrow_sb = sb("tmrow_sb", [1, 2 * EMB], F32)
        cfrow_sb = sb("cfrow_sb", [1, 2 * EMB], F16)
        coefr_sb = sb("coefr_sb", [128, 2 * EMB], F16)

        # psum: banks 0-5 window tiles; bank 6 out1/out2; bank 7 pool+stats
        wseg = [ctx.enter_context(nc.psum_tensor(f"wseg{i}", [128, 512], F32))
                for i in range(WG)]
        b6 = ctx.enter_context(nc.psum_tensor("b6", [128, 512], F32))
        out1_ps = [b6[:, 0:HID], b6[:, HID:2 * HID]]
        out2_ps = [b6[:, 256:256 + EMB], b6[:, 256 + EMB:256 + 2 * EMB]]
        b7 = ctx.enter_context(nc.psum_tensor("b7", [128, 512], F32))
        pool_ps = [b7[:, i * EMB:(i + 1) * EMB] for i in range(GHALF)]
        st_ps = b7[:1, 2 * EMB:4 * EMB]       # [1, 2D] row: sums | sqsums

        io = sem("io")
        iox = sem("iox")
        ioz = sem("ioz")
        gs = [sem(f"gs_{b}") for b in range(NBUF)]
        sdone = sem("sdone")
        pchunk = sem("pchunk")
        segcp = sem("segcp")
        w1d = sem("w1d")
        ocp = sem("ocp")            # ACT window out-tile + square copies
        stm = sem("stm")            # PE stats matmul pairs
        stc = sem("stc")            # ACT stats psum->sbuf row copies
        arS, ar2S = sem("arS"), sem("ar2S")
        agS = sem("agS")
        cc = sem("cc")
        ar1L, ar2L = sem("ar1L"), sem("ar2L")
        cfa, cfb = sem("cfa"), sem("cfb")
        cf1, cf2 = sem("cf1"), sem("cf2")
        cfr = sem("cfr")
        hap = sem("hap")
        bn2r = sem("bn2r")
        gG = sem("gG")
        plm = sem("plm")
        outc = sem("outc")
        iot = sem("iot")
        cfc = sem("cfc")
        ioh = sem("ioh")

        NLOAD = 8        # input dma_starts
        cfc_n = [0]

        def _chain(v, inst):
            cfc_n[0] += 1
            inst.then_inc(cfc, 1)
            v.wait_ge(cfc, cfc_n[0])

        def _coef_math(v, D, ar_sem, phase, gcol, becol):
            # strow [1, 2D] = AllReduced (sum | sqsum) of raw out over nodes.
            # a = g / sqrt(var+eps); bshift = be - a*mu  (conv bias is zero)
            # -> cfrow [1, 2D] f16 = (a | bshift)
            v.wait_ge(ar_sem, 16)
            _chain(v, v.tensor_scalar_mul(tmrow_sb[:, :2 * D],
                                          strow_sb[:, :2 * D], 1.0 / n_real))
            # strow[0:D] = mu^2 ; strow[D:2D] = var + eps
            _chain(v, v.tensor_tensor(out=strow_sb[:, 0:D],
                                      in0=tmrow_sb[:, 0:D],
                                      in1=tmrow_sb[:, 0:D], op=OP.mult))
            _chain(v, v.tensor_tensor(out=strow_sb[:, D:2 * D],
                                      in0=tmrow_sb[:, D:2 * D],
                                      in1=strow_sb[:, 0:D], op=OP.subtract))
            v.tensor_scalar_add(strow_sb[:, D:2 * D], strow_sb[:, D:2 * D],
                                EPS).then_inc(cfa, 1)
            v.wait_ge(cfb, phase)        # ACT took sqrt in place
            _chain(v, v.reciprocal(out=strow_sb[:, D:2 * D],
                                   in_=strow_sb[:, D:2 * D]))
            # a (f32) -> strow[0:D]
            _chain(v, v.tensor_tensor(out=strow_sb[:, 0:D],
                                      in0=strow_sb[:, D:2 * D],
                                      in1=bn_sb[:, gcol * 128:gcol * 128 + D],
                                      op=OP.mult))
            # a*mu -> tmrow[0:D]; bshift (f32) -> tmrow[D:2D]
            _chain(v, v.tensor_tensor(out=tmrow_sb[:, 0:D],
                                      in0=strow_sb[:, 0:D],
                                      in1=tmrow_sb[:, 0:D], op=OP.mult))
            _chain(v, v.tensor_tensor(out=tmrow_sb[:, D:2 * D],
                                      in0=bn_sb[:, becol * 128:becol * 128 + D],
                                      in1=tmrow_sb[:, 0:D], op=OP.subtract))
            # cast to f16 row (a | bshift)
            _chain(v, v.tensor_copy(out=cfrow_sb[:, 0:D],
                                    in_=strow_sb[:, 0:D]))
            v.tensor_copy(out=cfrow_sb[:, D:2 * D],
                          in_=tmrow_sb[:, D:2 * D]
                          ).then_inc(cf1 if phase == 1 else cf2, 1)

        with nc.Block() as block:

            # ------------------------------------------------ GPSIMD
            @block.gpsimd
            def _(gp: bass.BassGpSimd):
                gp.load_library(mlp)
                for dst_ap, src_ap in (
                    (idx_sb[:, :], idx1_d[:, :]),
                    (ew_sb[:, :], ew_d[:, :]),
                    (do1_sb[:, :], do1_d[:, :]),
                    (do2_sb[:, :], do2_d[:, :]),
                    (gid_sb[:, :], gid_d[:, :]),
                    (w1_sb[:, :], w1_d[:, :]),
                    (w2_sb[:, :], w2_d[:, :]),
                    (bn_sb[:, :], bn_d[:, :]),
                ):
                    gp.dma_start(dst_ap, src_ap).then_inc(io, 16)
                # zero gather buffers once (stale lanes must be finite)
                for i in range(NBUF):
                    gp.memset(mb[i][:, :, :], 0.0).then_inc(ioz, 1)
                gp.memset(ones_sb[:, :], 1.0).then_inc(ioz, 1)
                gp.wait_ge(ioz, NBUF + 1)
                gp.wait_ge(io, 16 * NLOAD)
                gp.iota(iota128[:, :], [[1, W]], base=0, channel_multiplier=0,
                        allow_small_or_imprecise_dtypes=True)
                gp.iota(iota256[:, :], [[1, 2 * W]], base=0,
                        channel_multiplier=0,
                        allow_small_or_imprecise_dtypes=True)
                gp.iota(iotg_sb[:, :], [[1, NG]], base=0, channel_multiplier=0,
                        allow_small_or_imprecise_dtypes=True).then_inc(iot, 1)

                # layer-1 gathers
                for ci, (cb, nch, r) in enumerate(calls):
                    q = ci % NQ
                    b = ci % NBUF
                    if ci >= NBUF:
                        gp.wait_ge(pchunk, cum_end[ci - NBUF])
                    nidx = nch * 128
                    gp.dma_gather(
                        mb[b][:, :nch, :], xp_d[r * RW:(r + 1) * RW, :],
                        idx_sb[:, cb * 8:cb * 8 + nidx // 16],
                        nidx, nidx, IN, queue_num=q,
                    ).then_inc(gs[b], 16)

                # BN1 stats AllReduce
                gp.wait_ge(arS, 16)
                gp.collective_compute(
                    "AllReduce", OP.add, replica_groups=[list(range(NC))],
                    ins=[ar1_in[:, :]], outs=[ar1_out[:, :]]).then_inc(cc, 1)

                # idx2 reload (L1 gathers all done once stats staged)
                gp.dma_start(idx_sb[:, :], idx2_d[:, :]).then_inc(iox, 16)

                # h AllGather
                gp.wait_ge(agS, 16)
                gp.collective_compute(
                    "AllGather", OP.bypass, replica_groups=[list(range(NC))],
                    ins=[ag_in[:, :]], outs=[ag_out[:, :]]).then_inc(cc, 1)

                # layer-2 gathers (h pairs from ag_out)
                gp.wait_ge(iox, 16)
                gp.wait_ge(cc, 2)
                for ci, (cb, nch, r) in enumerate(calls):
                    q = ci % NQ
                    b = ci % NBUF
                    if ci >= NBUF:
                        gp.wait_ge(pchunk, n_chunks + cum_end[ci - NBUF])
                    else:
                        gp.wait_ge(pchunk, n_chunks)
                    nidx = nch * 128
                    gp.dma_gather(
                        mb[b][:, :nch, :],
                        ag_out[r * RW:(r + 1) * RW, :]
                        .rearrange("(a b) d -> a (b d)", b=2),
                        idx_sb[:, cb * 8:cb * 8 + nidx // 16],
                        nidx, nidx, EMB, queue_num=q,
                    ).then_inc(gs[b], 16)

                # BN2 stats AllReduce
                gp.wait_ge(ar2S, 16)
                gp.collective_compute(
                    "AllReduce", OP.add, replica_groups=[list(range(NC))],
                    ins=[ar2_in[:, :]], outs=[ar2_out[:, :]]).then_inc(cc, 1)

            # ------------------------------------------------ VECTOR
            @block.vector
            def _(v):
                v.wait_ge(io, 16 * NLOAD)
                v.wait_ge(iot, 1)

                # L1 one-hot S
                for (cb, nch, r) in calls:
                    if cb + nch > NSLOT_S:
                        v.wait_ge(pchunk, cb + nch - NSLOT_S)
                    for t in range(cb, cb + nch):
                        v.tensor_scalar(
                            out=s_sb[t % NSLOT_S][:, :W], in0=iota128[:, :],
                            scalar1=do1_sb[:, t:t + 1],
                            scalar2=ew_sb[:, t:t + 1],
                            op0=OP.is_equal, op1=OP.mult).then_inc(sdone, 1)

                # BN1 coef (stats row AllReduced by now)
                _coef_math(v, HID, ar1L, 1, 1, 2)
                # BN1 apply per window on node-major tiles + ReLU
                v.wait_ge(cfr, 48)
                for wi in range(NWC):
                    w = worder[wi]
                    sl = slice(w * HID, (w + 1) * HID)
                    _chain(v, v.tensor_tensor(
                        out=out1h_sb[:, sl], in0=out1h_sb[:, sl],
                        in1=coefr_sb[:, 0:HID], op=OP.mult))
                    _chain(v, v.tensor_tensor(
                        out=out1h_sb[:, sl], in0=out1h_sb[:, sl],
                        in1=coefr_sb[:, EMB:EMB + HID], op=OP.add))
                    v.tensor_scalar_max(
                        out=out1h_sb[:, sl], in0=out1h_sb[:, sl],
                        scalar1=0.0).then_inc(hap, 1)

                # L2 one-hot S: narrow for single-parity chunks, 256-wide
                # (col = parity*128 + dstoff) for mixed chunks
                for (cb, nch, r) in calls:
                    v.wait_ge(pchunk, n_chunks + max(0, cb + nch - NSLOT_S))
                    for t in range(cb, cb + nch):
                        if chunk_kind[t] == 2:
                            v.tensor_scalar(
                                out=s_sb[t % NSLOT_S][:, :], in0=iota256[:, :],
                                scalar1=do2_sb[:, t:t + 1],
                                scalar2=ew_sb[:, t:t + 1],
                                op0=OP.is_equal, op1=OP.mult).then_inc(sdone, 1)
                        else:
                            v.tensor_scalar(
                                out=s_sb[t % NSLOT_S][:, :W], in0=iota128[:, :],
                                scalar1=do1_sb[:, t:t + 1],
                                scalar2=ew_sb[:, t:t + 1],
                                op0=OP.is_equal, op1=OP.mult).then_inc(sdone, 1)

                # BN2 coef
                _coef_math(v, EMB, ar2L, 2, 4, 5)
                # BN2 apply per window + ReLU; gone one-hots
                v.wait_ge(cfr, 80)
                for wi in range(NWC):
                    w = worder[wi]
                    sl = slice(w * EMB, (w + 1) * EMB)
                    _chain(v, v.tensor_tensor(
                        out=out2_sb[:, sl], in0=out2_sb[:, sl],
                        in1=coefr_sb[:, 0:EMB], op=OP.mult))
                    _chain(v, v.tensor_tensor(
                        out=out2_sb[:, sl], in0=out2_sb[:, sl],
                        in1=coefr_sb[:, EMB:2 * EMB], op=OP.add))
                    v.tensor_scalar_max(
                        out=out2_sb[:, sl], in0=out2_sb[:, sl],
                        scalar1=0.0).then_inc(bn2r, 1)
                    if wi >= 2:
                        v.wait_ge(plm, wi - 1)
                    v.tensor_scalar(
                        out=gone_sb[wi % 2][:, :], in0=iotg_sb[:, :],
                        scalar1=gid_sb[:, w:w + 1], scalar2=None,
                        op0=OP.is_equal).then_inc(gG, 1)

            # ------------------------------------------------ SCALAR
            @block.scalar
            def _(sc):
                sc.wait_ge(io, 16 * NLOAD)
                for L in range(2):
                    D = HID if L == 0 else EMB
                    osb = out1h_sb if L == 0 else out2_sb
                    ops = out1_ps if L == 0 else out2_ps
                    # per-window drain: seg copy (for W matmul), out copy + sq
                    for wi in range(NWC):
                        w = worder[wi]
                        sc.wait_ge(pchunk, L * n_chunks + wlast[w] + 1)
                        if wi >= 2:
                            sc.wait_ge(w1d, L * NWC + wi - 1)
                        if L == 0:
                            sc.activation(out=seg_sb[wi % 2][:, :],
                                          in_=wseg[wi % WG][:, :W],
                                          func=AF.Copy).then_inc(segcp, 1)
                        else:
                            sc.activation(out=seg_sb[wi % 2][:HID, :],
                                          in_=wseg[wi % WG][:HID, :W],
                                          func=AF.Copy).then_inc(segcp, 1)
                        sc.wait_ge(w1d, L * NWC + wi + 1)
                        if wi >= 2:
                            sc.wait_ge(stm, L * NWC + wi - 1)
                        sc.activation(out=osb[:, w * D:(w + 1) * D],
                                      in_=ops[wi % 2][:, :D], func=AF.Copy)
                        sc.activation(out=sq_sb[wi % 2][:, :D],
                                      in_=ops[wi % 2][:, :D],
                                      func=AF.Square).then_inc(ocp, 1)
                    # stats row psum -> sbuf
                    sc.wait_ge(stm, (L + 1) * NWC)
                    sc.activation(out=strow_sb[:, :2 * D],
                                  in_=st_ps[:, :2 * D],
                                  func=AF.Copy).then_inc(stc, 1)
                    # sqrt for BN coef
                    sc.wait_ge(cfa, L + 1)
                    sc.activation(out=strow_sb[:, D:2 * D],
                                  in_=strow_sb[:, D:2 * D],
                                  func=AF.Sqrt).then_inc(cfb, 1)
                # final pool copy
                sc.wait_ge(plm, NWC)
                sc.activation(out=pout_sb[:, :], in_=b7[:, :GHALF * EMB],
                              func=AF.Copy).then_inc(outc, 1)

            # ------------------------------------------------ TENSOR
            @block.tensor
            def _(pe):
                pe.wait_ge(io, 16 * NLOAD)
                uses = [0] * NBUF

                for L in range(2):
                    D = HID if L == 0 else EMB
                    osb = out1h_sb if L == 0 else out2_sb
                    ops = out1_ps if L == 0 else out2_ps
                    done_w = 0
                    done_st = 0

                    def drain(upto_w, upto_st, L=L, D=D, osb=osb, ops=ops):
                        nonlocal done_w, done_st
                        while done_w < upto_w:
                            wi = done_w
                            pe.wait_ge(segcp, L * NWC + wi + 1)
                            if wi >= 1:
                                pe.wait_ge(ocp, L * NWC + wi)
                            if L == 0:
                                pe.matmul(ops[wi % 2][:, :HID],
                                          lhsT=seg_sb[wi % 2][:, :],
                                          rhs=w1_sb[:, :],
                                          start=True, stop=True
                                          ).then_inc(w1d, 1)
                            else:
                                pe.matmul(ops[wi % 2][:, :EMB],
                                          lhsT=seg_sb[wi % 2][:HID, :],
                                          rhs=w2_sb[:, :],
                                          start=True, stop=True
                                          ).then_inc(w1d, 1)
                            done_w += 1
                        while done_st < upto_st:
                            wi = done_st
                            w = worder[wi]
                            pe.wait_ge(ocp, L * NWC + wi + 1)
                            pe.matmul(st_ps[:, 0:D],
                                      lhsT=ones_sb[:, :],
                                      rhs=osb[:, w * D:(w + 1) * D],
                                      start=(wi == 0 and L == 0) or
                                            (wi == 0 and L == 1),
                                      stop=False)
                            pe.matmul(st_ps[:, D:2 * D],
                                      lhsT=ones_sb[:, :],
                                      rhs=sq_sb[wi % 2][:, :D],
                                      start=False,
                                      stop=(wi == NWC - 1)).then_inc(stm, 1)
                            done_st += 1

                    seen_w = 0
                    for ci, (cb, nch, r) in enumerate(calls):
                        b = ci % NBUF
                        uses[b] += 1
                        pe.wait_ge(gs[b], 16 * uses[b])
                        pe.wait_ge(sdone, L * n_chunks + cb + nch)
                        for k in range(nch):
                            t = cb + k
                            w = chunk_window[t]
                            wi = wpos[w]
                            if t == wfirst[w] and wi >= WG:
                                pe.wait_ge(segcp, L * NWC + wi - WG + 1)
                            first, lastc = t == wfirst[w], t == wlast[w]
                            if L == 0:
                                pe.matmul(wseg[wi % WG][:, :W],
                                          lhsT=mb[b][:, k, :],
                                          rhs=s_sb[t % NSLOT_S][:, :W],
                                          start=first,
                                          stop=lastc).then_inc(pchunk, 1)
                            elif chunk_kind[t] == 2:
                                pe.matmul(wseg[wi % WG][:HID, :W],
                                          lhsT=mb[b][:, k, 0:HID],
                                          rhs=s_sb[t % NSLOT_S][:, 0:W],
                                          start=first, stop=False)
                                pe.matmul(wseg[wi % WG][:HID, :W],
                                          lhsT=mb[b][:, k, HID:2 * HID],
                                          rhs=s_sb[t % NSLOT_S][:, W:2 * W],
                                          start=False,
                                          stop=lastc).then_inc(pchunk, 1)
                            else:
                                hh = chunk_kind[t] * HID
                                pe.matmul(wseg[wi % WG][:HID, :W],
                                          lhsT=mb[b][:, k, hh:hh + HID],
                                          rhs=s_sb[t % NSLOT_S][:, :W],
                                          start=first,
                                          stop=lastc).then_inc(pchunk, 1)
                            if lastc:
                                seen_w += 1
                                drain(max(0, seen_w - 1), max(0, seen_w - 2))
                    drain(NWC, NWC)

                # pool matmuls on BN2-applied node-major tiles
                for wi in range(NWC):
                    w = worder[wi]
                    pe.wait_ge(bn2r, wi + 1)
                    pe.wait_ge(gG, wi + 1)
                    for gh in range(GHALF):
                        mm = pe.matmul(
                            pool_ps[gh][:, :],
                            lhsT=gone_sb[wi % 2][:, gh * 128:(gh + 1) * 128],
                            rhs=out2_sb[:, w * EMB:(w + 1) * EMB],
                            start=(wi == 0 and gh == 0),
                            stop=(wi == NWC - 1 and gh == GHALF - 1))
                        if gh == GHALF - 1:
                            mm.then_inc(plm, 1)

            # ------------------------------------------------ SYNC
            @block.sync
            def _(sy):
                # BN1 stats row out / in
                sy.wait_ge(stc, 1)
                sy.dma_start(ar1_in[:, :], strow_sb[:, :2 * HID]).then_inc(arS, 16)
                sy.wait_ge(cc, 1)
                sy.dma_start(strow_sb[:, :2 * HID], ar1_out[:, :]).then_inc(ar1L, 16)
                # BN1 coef row -> DRAM -> replicated rows
                sy.wait_ge(cf1, 1)
                sy.dma_start(cf1row_d[:, :], cfrow_sb[:, :2 * HID]).then_inc(cfr, 16)
                sy.wait_ge(cfr, 16)
                rep1 = bass.AP(cf1row_d, 0, [[0, 128], [1, 2 * HID]])
                sy.dma_start(coefr_sb[:, 0:HID], rep1[:, 0:HID]).then_inc(cfr, 16)
                sy.dma_start(coefr_sb[:, EMB:EMB + HID],
                             rep1[:, HID:2 * HID]).then_inc(cfr, 16)
                # h to AllGather input (BN1-applied, node-major)
                sy.wait_ge(hap, NWC)
                sy.dma_start(
                    ag_in[:, :].rearrange("(w p) c -> p w c", p=128),
                    out1h_sb[:, :].rearrange("p (w c) -> p w c", c=HID),
                ).then_inc(agS, 16)
                # BN2 stats row
                sy.wait_ge(stc, 2)
                sy.dma_start(ar2_in[:, :], strow_sb[:, :2 * EMB]).then_inc(ar2S, 16)
                sy.wait_ge(cc, 3)
                sy.dma_start(strow_sb[:, :2 * EMB], ar2_out[:, :]).then_inc(ar2L, 16)
                # BN2 coef row -> DRAM -> replicated rows
                sy.wait_ge(cf2, 1)
                sy.dma_start(cf2row_d[:, :], cfrow_sb[:, :2 * EMB]).then_inc(cfr, 16)
                sy.wait_ge(cfr, 64)
                rep2 = bass.AP(cf2row_d, 0, [[0, 128], [1, 2 * EMB]])
                sy.dma_start(coefr_sb[:, :], rep2[:, :]).then_inc(cfr, 16)
                # final output
                sy.wait_ge(outc, 1)
                sy.dma_start(
                    out_d[:, :, :].rearrange("g p d -> p g d"),
                    pout_sb[:, :].rearrange("p (g d) -> p g d", d=EMB),
                ).then_inc(ioh, 16)
                sy.wait_ge(ioh, 16)

    nc.compile()
    return nc


# ==================================================================== entry
def _make_in_maps(inputs, cfg, percore, xp16):
    HID, EMB = cfg["hid_dim"], cfg["emb_dim"]
    bnp = np.zeros((1, 6 * 128), np.float32)
    bnp[0, 0:HID] = np.asarray(inputs["b1"], np.float32)
    bnp[0, 128:128 + HID] = np.asarray(inputs["g1"], np.float32)
    bnp[0, 256:256 + HID] = np.asarray(inputs["be1"], np.float32)
    bnp[0, 384:384 + EMB] = np.asarray(inputs["b2"], np.float32)
    bnp[0, 512:512 + EMB] = np.asarray(inputs["g2"], np.float32)
    bnp[0, 640:640 + EMB] = np.asarray(inputs["be2"], np.float32)
    w1 = np.asarray(inputs["W1"], np.float32).astype(np.float16)
    w2 = np.asarray(inputs["W2"], np.float32).astype(np.float16)
    return [dict(
        xp16=xp16, idx1=percore["idx1"][c], idx2=percore["idx2"][c],
        ewt=percore["ew"][c], dof1=percore["dof1"][c],
        dof2=percore["dof2"][c], gid=percore["gid"][c],
        w1=w1, w2=w2, bnp=bnp,
    ) for c in range(cfg["n_cores"])]


def _run(inputs, cfg):
    x = np.asarray(inputs["x"], np.float32)
    layout, percore, xp16, slot = _host_prep(
        x, inputs["edge_index"], inputs["edge_weight"], inputs["batch_vec"], cfg)
    nc = _build(cfg, layout)

    NC = cfg["n_cores"]
    in_maps = _make_in_maps(inputs, cfg, percore, xp16)
    res = None
    last_err = None
    for attempt in range(3):
        try:
            res = run_bass_kernel_spmd(nc, in_maps, list(range(NC)), trace=TRACE)
            break
        except Exception as err:      # wedged device: retry
            last_err = err
    if res is None:
        raise last_err

    NG, EMB = cfg["n_graphs"], cfg["emb_dim"]
    pool = np.zeros((NG, EMB), np.float64)
    for c in range(NC):
        p = res.results[c]["pool"].astype(np.float64)   # [GHALF, 128, EMB]
        pool += p.reshape(NG, EMB)
    counts = np.bincount(np.asarray(inputs["batch_vec"], np.int64),
                         minlength=NG).astype(np.float64)
    pool /= np.maximum(counts, 1.0)[:, None]
    return pool.astype(np.float32), res


def kernel(**inputs):
    out, _ = _run(inputs, CFG_FULL)
    return out
